# revision 4
# baseline (speedup 1.0000x reference)
"""Trainium2 Bass kernel for nn_Actor (MLP 17->400->300->6 + tanh + weighted-L1-ball
projection), data-parallel over 8 NeuronCores.

Layout strategy (per core, BL = 32768 rows):
  - state is host-transposed to [17, BL] (feature-major) so the 17-dim contraction
    sits on PE partitions with rows streaming as the matmul free dim.
  - L1/L2 keep features on PSUM partitions (chunks of 100), rows free (N=512).
  - L3 flips to rows-on-partitions by using h2T slices as the (transposed) stationary
    operand, giving [128 rows, 6] outputs directly in the projection's natural layout.
  - The weighted-L1 projection runs as a division-free Newton/active-set iteration on
    [128, 256, 6] tiles: lambda is tracked as a pair (Ac, B) with the active-set test
    aq*B > Ac*w, which converges exactly in <= 6 iterations for D=6.
All matmuls use float32r (PE-rounded fp32): ~2e-4 relative error, 1 col/cycle.
"""

import numpy as np

import concourse.bass as bass
from concourse import mybir
from concourse.bass_utils import run_bass_kernel_spmd

F32 = mybir.dt.float32
F32R = mybir.dt.float32r
AluOp = mybir.AluOpType
ActFn = mybir.ActivationFunctionType

NCORES = 8
B_TOTAL = 262144
SD = 17          # state dim
AD = 6           # action dim
H1 = 400
H2 = 300
BL = B_TOTAL // NCORES   # rows per core
RG = 512                 # rows per group (matmul free dim)
G = BL // RG             # groups per core
NR = BL // 128           # 128-row blocks per core
CHUNK_GROUPS = 16        # state streaming chunk = 16 groups
NCHUNK = G // CHUNK_GROUPS
C_RADIUS = 20.0
N_ITERS = 6              # projection active-set iterations (exact for D=6)


def _build():
    nc = bass.Bass("TRN2", target_bir_lowering=False)

    st_d = nc.dram_tensor("state_t", [SD, BL], F32R, kind="ExternalInput")
    wr_d = nc.dram_tensor("w_rows", [128, NR, AD], F32, kind="ExternalInput")
    w1_d = nc.dram_tensor("w1t", [SD, 4, 100], F32R, kind="ExternalInput")
    b1_d = nc.dram_tensor("b1", [100, 4], F32, kind="ExternalInput")
    w2_d = nc.dram_tensor("w2t", [100, 4, H2], F32R, kind="ExternalInput")
    b2_d = nc.dram_tensor("b2", [100, 3], F32, kind="ExternalInput")
    w3_d = nc.dram_tensor("w3t", [100, 3, AD], F32R, kind="ExternalInput")
    b3_d = nc.dram_tensor("b3", [128, AD], F32, kind="ExternalInput")
    out_d = nc.dram_tensor("out_rows", [128, NR, AD], F32, kind="ExternalOutput")

    CR = CHUNK_GROUPS * RG   # rows per state chunk

    from contextlib import ExitStack
    with ExitStack() as ctx:
        s_dma = ctx.enter_context(nc.semaphore("s_dma"))
        s_st = ctx.enter_context(nc.semaphore("s_st"))
        s_pe1 = ctx.enter_context(nc.semaphore("s_pe1"))
        s_pe2 = ctx.enter_context(nc.semaphore("s_pe2"))
        s_pe3 = ctx.enter_context(nc.semaphore("s_pe3"))
        s_ev1 = ctx.enter_context(nc.semaphore("s_ev1"))
        s_ev2 = ctx.enter_context(nc.semaphore("s_ev2"))
        s_q = ctx.enter_context(nc.semaphore("s_q"))
        s_tanh = ctx.enter_context(nc.semaphore("s_tanh"))
        s_proj = ctx.enter_context(nc.semaphore("s_proj"))
        st_s = ctx.enter_context(nc.sbuf_tensor("st_s", [SD, 2, CR], F32R))
        wr_s = ctx.enter_context(nc.sbuf_tensor("wr_s", [128, NR, AD], F32))
        w1_s = ctx.enter_context(nc.sbuf_tensor("w1_s", [SD, 4, 100], F32R))
        b1_s = ctx.enter_context(nc.sbuf_tensor("b1_s", [100, 4], F32))
        w2_s = ctx.enter_context(nc.sbuf_tensor("w2_s", [100, 4, H2], F32R))
        b2_s = ctx.enter_context(nc.sbuf_tensor("b2_s", [100, 3], F32))
        w3_s = ctx.enter_context(nc.sbuf_tensor("w3_s", [100, 3, AD], F32R))
        b3_s = ctx.enter_context(nc.sbuf_tensor("b3_s", [128, AD], F32))
        h1_s = ctx.enter_context(nc.sbuf_tensor("h1_s", [100, 2, 4, RG], F32R))
        h2_s = ctx.enter_context(nc.sbuf_tensor("h2_s", [100, 2, 3, RG], F32R))
        q_s = ctx.enter_context(nc.sbuf_tensor("q_s", [128, NR, AD], F32))
        aq_s = ctx.enter_context(nc.sbuf_tensor("aq_s", [128, NR, AD], F32))
        wa_s = ctx.enter_context(nc.sbuf_tensor("wa_s", [128, NR, AD], F32))
        ww_s = ctx.enter_context(nc.sbuf_tensor("ww_s", [128, NR, AD], F32))
        t1_s = ctx.enter_context(nc.sbuf_tensor("t1_s", [128, NR, AD], F32))
        t2_s = ctx.enter_context(nc.sbuf_tensor("t2_s", [128, NR, AD], F32))
        A_s = ctx.enter_context(nc.sbuf_tensor("A_s", [128, NR], F32))
        Bv_s = ctx.enter_context(nc.sbuf_tensor("Bv_s", [128, NR], F32))
        Ac_s = ctx.enter_context(nc.sbuf_tensor("Ac_s", [128, NR], F32))
        p1 = ctx.enter_context(nc.psum_tensor("p1", [100, 4, RG], F32))
        p2 = ctx.enter_context(nc.psum_tensor("p2", [100, 3, RG], F32))
        p3 = ctx.enter_context(nc.psum_tensor("p3", [128, 4, AD], F32))
        block = ctx.enter_context(nc.Block())

        @block.sync
        def _(sp):
            for dst, src in (
                (w1_s, w1_d), (b1_s, b1_d), (w2_s, w2_d), (b2_s, b2_d),
                (w3_s, w3_d), (b3_s, b3_d), (wr_s, wr_d),
            ):
                sp.dma_start(out=dst[:], in_=src[:]).then_inc(s_dma, 16)
            for k in range(NCHUNK):
                if k >= 1:
                    # serialize chunk completions: HW queues can finish out of
                    # order, and PE's per-chunk thresholds assume chunk order
                    sp.wait_ge(s_st, 16 * k)
                if k >= 2:
                    # buffer k%2 free once L1 has consumed chunk k-2
                    sp.wait_ge(s_pe1, CHUNK_GROUPS * (k - 1))
                sp.dma_start(
                    out=st_s[:, k % 2, :], in_=st_d[:, k * CR:(k + 1) * CR]
                ).then_inc(s_st, 16)

        @block.tensor
        def _(pe):
            pe.wait_ge(s_dma, 16 * 7)
            for i in range(G + 2):
                # ---- L1(i) ----
                if i < G:
                    if i % CHUNK_GROUPS == 0:
                        pe.wait_ge(s_st, 16 * (i // CHUNK_GROUPS + 1))
                    if i >= 1:
                        pe.wait_ge(s_ev1, i)  # psum1 free (evac1(i-1) done)
                    rhs = st_s[:, (i // CHUNK_GROUPS) % 2,
                               (i % CHUNK_GROUPS) * RG:(i % CHUNK_GROUPS + 1) * RG]
                    for c in range(4):
                        mm = pe.matmul(p1[:, c, :], w1_s[:, c, :], rhs,
                                       start=True, stop=True)
                    mm.then_inc(s_pe1, 1)
                # ---- L2(i-1) ----
                if 1 <= i <= G:
                    g = i - 1
                    pe.wait_ge(s_ev1, g + 1)
                    if g >= 1:
                        pe.wait_ge(s_ev2, g)  # psum2 free (evac2(g-1) done)
                    for mc in range(3):
                        for kc in range(4):
                            mm = pe.matmul(
                                p2[:, mc, :],
                                w2_s[:, kc, mc * 100:(mc + 1) * 100],
                                h1_s[:, g % 2, kc, :],
                                start=(kc == 0), stop=(kc == 3),
                            )
                    mm.then_inc(s_pe2, 1)
                # ---- L3(i-2) ----
                if 2 <= i <= G + 1:
                    g = i - 2
                    pe.wait_ge(s_ev2, g + 1)
                    if g >= 1:
                        pe.wait_ge(s_q, g)  # psum3 free (q-add(g-1) done)
                    for t2 in range(4):
                        for kc in range(3):
                            mm = pe.matmul(
                                p3[:, t2, :],
                                h2_s[:, g % 2, kc, t2 * 128:(t2 + 1) * 128],
                                w3_s[:, kc, :],
                                start=(t2 == 0 and kc == 0),
                                stop=(t2 == 3 and kc == 2),
                                skip_group_check=True,
                            )
                    mm.then_inc(s_pe3, 1)

        @block.scalar
        def _(act):
            act.wait_ge(s_dma, 16 * 7)
            for i in range(G + 1):
                # ---- evac1(i): h1 = relu(p1 + b1) ----
                if i < G:
                    act.wait_ge(s_pe1, i + 1)
                    if i >= 2:
                        act.wait_ge(s_pe2, i - 1)  # h1 buf free (L2(i-2) done)
                    for c in range(4):
                        a = act.activation(
                            out=h1_s[:, i % 2, c, :], in_=p1[:, c, :],
                            func=ActFn.Relu, bias=b1_s[:, c:c + 1], scale=1.0,
                        )
                    a.then_inc(s_ev1, 1)
                # ---- evac2(i-1): h2 = relu(p2 + b2) ----
                if i >= 1:
                    g = i - 1
                    act.wait_ge(s_pe2, g + 1)
                    if g >= 2:
                        act.wait_ge(s_pe3, g - 1)  # h2 buf free (L3(g-2) done)
                    for mc in range(3):
                        a = act.activation(
                            out=h2_s[:, g % 2, mc, :], in_=p2[:, mc, :],
                            func=ActFn.Relu, bias=b2_s[:, mc:mc + 1], scale=1.0,
                        )
                    a.then_inc(s_ev2, 1)
            # ---- tanh + abs over the whole shard ----
            act.wait_ge(s_q, G)
            act.activation(out=q_s[:], in_=q_s[:], func=ActFn.Tanh).then_inc(s_tanh, 1)
            act.activation(out=aq_s[:], in_=q_s[:], func=ActFn.Abs).then_inc(s_tanh, 1)

        @block.vector
        def _(dve):
            dve.wait_ge(s_dma, 16 * 7)
            b3b = b3_s[:, None, :].to_broadcast((128, 4, AD))
            for g in range(G):
                dve.wait_ge(s_pe3, g + 1)
                dve.tensor_tensor(
                    q_s[:, 4 * g:4 * (g + 1), :], p3[:], b3b, AluOp.add
                ).then_inc(s_q, 1)
            # ---- projection (whole shard, [128, NR, 6]) ----
            # lam_{k+1} = max(lam_k, relu(A_k - c) / B_k) with the active set
            # {i : aq_i > lam_k * w_i}; monotone in exact arithmetic, and the
            # max() makes it robust to mask flips from matmul rounding noise.
            dve.wait_ge(s_tanh, 2)
            dve.tensor_tensor(wa_s[:], wr_s[:], aq_s[:], AluOp.mult)
            dve.tensor_tensor(ww_s[:], wr_s[:], wr_s[:], AluOp.mult)
            dve.memset(Ac_s[:], 0.0)  # Ac holds lambda
            AcB = Ac_s[:, :, None].to_broadcast((128, NR, AD))
            for it in range(N_ITERS):
                dve.tensor_tensor(t2_s[:], wr_s[:], AcB, AluOp.mult)   # lam*w
                dve.tensor_tensor(t1_s[:], aq_s[:], t2_s[:], AluOp.is_gt)
                dve.tensor_tensor(t2_s[:], t1_s[:], wa_s[:], AluOp.mult)
                dve.tensor_reduce(A_s[:], t2_s[:], axis=mybir.AxisListType.X,
                                  op=AluOp.add)
                dve.tensor_tensor(t2_s[:], t1_s[:], ww_s[:], AluOp.mult)
                dve.tensor_reduce(Bv_s[:], t2_s[:], axis=mybir.AxisListType.X,
                                  op=AluOp.add)
                dve.tensor_scalar_add(Bv_s[:], Bv_s[:], 1e-30)
                dve.reciprocal(Bv_s[:], Bv_s[:])
                dve.tensor_scalar(A_s[:], A_s[:], C_RADIUS, 0.0,
                                  op0=AluOp.subtract, op1=AluOp.max)
                dve.tensor_tensor(A_s[:], A_s[:], Bv_s[:], AluOp.mult)
                dve.tensor_tensor(Ac_s[:], Ac_s[:], A_s[:], AluOp.max)
            # x = q - clip(q, -lambda*w, lambda*w)
            dve.tensor_tensor(t2_s[:], wr_s[:], AcB, AluOp.mult)       # lw
            dve.tensor_tensor(t1_s[:], q_s[:], t2_s[:], AluOp.min)     # min(q, lw)
            dve.scalar_tensor_tensor(t1_s[:], t2_s[:], -1.0, t1_s[:],
                                     op0=AluOp.mult, op1=AluOp.max)    # clip
            dve.tensor_tensor(q_s[:], q_s[:], t1_s[:],
                              AluOp.subtract).then_inc(s_proj, 1)

        @block.sync
        def _(sp):
            sp.wait_ge(s_proj, 1)
            sp.dma_start(out=out_d[:], in_=q_s[:]).then_inc(s_dma, 16)
            sp.wait_ge(s_dma, 16 * 8)

    return nc


_NC_CACHE = None


def _get_nc():
    global _NC_CACHE
    if _NC_CACHE is None:
        _NC_CACHE = _build()
    return _NC_CACHE


def kernel(state, l1_w, l1_b, l2_w, l2_b, l3_w, l3_b, training=0, **_unused):
    state = np.ascontiguousarray(np.asarray(state, np.float32))
    l1_w = np.asarray(l1_w, np.float32)
    l1_b = np.asarray(l1_b, np.float32)
    l2_w = np.asarray(l2_w, np.float32)
    l2_b = np.asarray(l2_b, np.float32)
    l3_w = np.asarray(l3_w, np.float32)
    l3_b = np.asarray(l3_b, np.float32)

    w1t = np.ascontiguousarray(l1_w.reshape(4, 100, SD).transpose(2, 0, 1))
    b1 = np.ascontiguousarray(l1_b.reshape(4, 100).T)
    w2t = np.ascontiguousarray(l2_w.T.reshape(4, 100, H2).transpose(1, 0, 2))
    b2 = np.ascontiguousarray(l2_b.reshape(3, 100).T)
    w3t = np.ascontiguousarray(l3_w.T.reshape(3, 100, AD).transpose(1, 0, 2))
    b3 = np.ascontiguousarray(np.broadcast_to(l3_b, (128, AD)))

    in_maps = []
    for i in range(NCORES):
        shard = state[i * BL:(i + 1) * BL]
        st_t = np.ascontiguousarray(shard.T)
        w_rows = np.ascontiguousarray(
            np.abs(shard[:, 11:17]).reshape(NR, 128, AD).transpose(1, 0, 2)
        )
        in_maps.append({
            "state_t": st_t, "w_rows": w_rows,
            "w1t": w1t, "b1": b1, "w2t": w2t, "b2": b2,
            "w3t": w3t, "b3": b3,
        })

    res = run_bass_kernel_spmd(_get_nc(), in_maps, core_ids=list(range(NCORES)))
    outs = [
        res.results[i]["out_rows"].transpose(1, 0, 2).reshape(BL, AD)
        for i in range(NCORES)
    ]
    return np.ascontiguousarray(np.concatenate(outs, axis=0).astype(np.float32))
